# revision 16
# baseline (speedup 1.0000x reference)
"""Bahdanau-style attention kernel for Trainium2, 8-way data-parallel over batch.

reference math (per batch b):
  w1_ah  = hidden @ W1 + W1_b                      [B, A]
  w2_hs  = features @ W2 + W2_b                    [B, L, A]
  combined = tanh(w2_hs + w1_ah[:, None, :])       [B, L, A]
  scores = combined @ A_w + A_b                    [B, L]
  alpha  = softmax(scores, axis=1)                 [B, L]
  context = einsum('ble,bl->be', features, alpha)  [B, E]
  returns (alpha, context)

Sharding: batch 128 -> 16 per core x 8 cores. Weights replicated.

Device-side layout (per core, T = 16*196 = 3136 tokens):
  - Host pre-transposes/pre-casts features to bf16 in two layouts:
      xt [128, 16, 3136]: partition p, (ec, t) -> features[t, ec*128+p]
      xn [3136, 2048]   : natural token-major (for the context matmul)
  - Main matmul: stationary W2 tile [128e, 128a], moving xt slice [128e, 512t]
    -> PSUM [128a, 512t] accumulated over 16 e-chunks ("combined^T" layout).
  - tanh on ScalarE with per-partition bias = (hidden@W1 + W1_b + W2_b)^T column.
  - scores: stationary A_w chunk [128a, 1], moving combined^T -> [1, 512t],
    landed into a [16, 196] scores tile via small per-segment SBUF DMAs.
  - softmax + context are pipelined in batch groups under the main loop:
    as soon as all scores of a batch group are done, softmax that group,
    transpose its alpha rows on PE, mask per-batch, and run its context
    matmuls while later t-tiles are still streaming.
"""

import numpy as np
import ml_dtypes

import concourse.bacc as bacc
import concourse.bass as bass
import concourse.mybir as mybir
import concourse.tile as tile
from concourse.bass_utils import run_bass_kernel_spmd

BF16 = ml_dtypes.bfloat16
F32 = np.float32

N_CORES = 8
B_FULL, L, E, D, A = 128, 196, 2048, 512, 512
BPC = B_FULL // N_CORES            # 16 batches per core
T = BPC * L                        # 3136 tokens per core
EC = E // 128                      # 16 e-chunks
AT = A // 128                      # 4 a-tiles
DC = D // 128                      # 4 d-chunks
TW = 512                           # t-tile width for the main matmul
T_TILES = [(t0, min(TW, T - t0)) for t0 in range(0, T, TW)]

# batch groups for pipelined softmax+context: (batches, ready_after_tile_idx)
#   group is ready when (last_b+1)*196 <= end of that t-tile
B_GROUPS = [
    (list(range(0, 6)), 2),     # 6*196=1176  <= 1536
    (list(range(6, 11)), 4),    # 11*196=2156 <= 2560
    (list(range(11, 14)), 5),   # 14*196=2744 <= 3072
    (list(range(14, 16)), 6),   # 16*196=3136 <= 3136
]
# xn DMA schedule: which batches' xn tiles to enqueue after each t-tile's xt DMA
XN_SCHED = {1: [0, 1, 2], 2: [3, 4, 5], 3: [6, 7, 8],
            4: [9, 10, 11], 5: [12, 13, 14], 6: [15]}

fp32 = mybir.dt.float32
bf16 = mybir.dt.bfloat16


def _segments(t0, w):
    """Yield (b, s0, s1) batch-constant segments of [t0, t0+w) in local coords."""
    t = t0
    end = t0 + w
    while t < end:
        b = t // L
        seg_end = min(end, (b + 1) * L)
        yield b, t - t0, seg_end - t0
        t = seg_end


def _build_program():
    nc = bacc.Bacc("TRN2")

    xt = nc.dram_tensor("xt", [128, EC, T], bf16, kind="ExternalInput")
    xn = nc.dram_tensor("xn", [T, E], bf16, kind="ExternalInput")
    w2s = nc.dram_tensor("w2s", [128, EC, A], bf16, kind="ExternalInput")
    w1s = nc.dram_tensor("w1s", [128, DC, A], fp32, kind="ExternalInput")
    hid = nc.dram_tensor("hid", [128, DC, BPC], fp32, kind="ExternalInput")
    b12 = nc.dram_tensor("b12", [128, AT], fp32, kind="ExternalInput")
    aw = nc.dram_tensor("aw", [128, AT], bf16, kind="ExternalInput")
    msk = nc.dram_tensor("msk", [128, BPC * BPC], bf16, kind="ExternalInput")
    id16 = nc.dram_tensor("id16", [BPC, BPC], fp32, kind="ExternalInput")

    alpha_o = nc.dram_tensor("alpha", [BPC, L], fp32, kind="ExternalOutput")
    ctx_o = nc.dram_tensor("ctx", [BPC, E], fp32, kind="ExternalOutput")

    with tile.TileContext(nc) as tc:
        with (
            tc.tile_pool(name="const", bufs=1) as constp,
            tc.tile_pool(name="xtp", bufs=2) as xtp,
            tc.tile_pool(name="xnp", bufs=1) as xnp,
            tc.tile_pool(name="cmb", bufs=2) as cmbp,
            tc.tile_pool(name="small", bufs=1) as smallp,
            tc.tile_pool(name="atp", bufs=1) as atp,
            tc.tile_pool(name="psmm", bufs=2, space="PSUM") as psmm,
            tc.tile_pool(name="pssc", bufs=2, space="PSUM") as pssc,
            tc.tile_pool(name="psctx", bufs=1, space="PSUM") as psctx,
        ):
            # ---- small consts first (cheap DMAs so stage 0 starts early) ----
            B12 = constp.tile([128, AT], fp32)
            nc.sync.dma_start(B12[:], b12[:])
            AW = constp.tile([128, AT], bf16)
            nc.sync.dma_start(AW[:], aw[:])
            MSK = constp.tile([128, BPC * BPC], bf16)
            nc.sync.dma_start(MSK[:], msk[:])
            ID16 = constp.tile([BPC, BPC], fp32)
            nc.sync.dma_start(ID16[:], id16[:])

            # ---- stage 0: w1b[a, at, b] = (hidden @ W1)^T + (W1_b + W2_b) ----
            w1b = smallp.tile([128, AT, BPC], fp32)
            if True:
                W1s = xtp.tile([128, DC, A], fp32, tag="xt")
                nc.sync.dma_start(W1s[:], w1s[:])
                Hs = xtp.tile([128, DC, BPC], fp32, tag="xt")
                nc.sync.dma_start(Hs[:], hid[:])
                for at in range(AT):
                    ps_w = psmm.tile([128, BPC], fp32, tag="mm", name=f"ps_w{at}")
                    for dc in range(DC):
                        nc.tensor.matmul(
                            ps_w[:],
                            W1s[:, dc, at * 128:(at + 1) * 128],
                            Hs[:, dc, :],
                            start=(dc == 0),
                            stop=(dc == DC - 1),
                        )
                    nc.vector.tensor_scalar_add(
                        w1b[:, at, :], ps_w[:], B12[:, at:at + 1])

            # ---- big weights ----
            W2s = constp.tile([128, EC, A], bf16)
            nc.sync.dma_start(W2s[:], w2s[:])

            # persistent small tiles for scores/softmax/context (one set per
            # batch group, all based at partition 0 -- engine ops cannot
            # address partition ranges at arbitrary offsets)
            GN = [len(bs) for bs, _ in B_GROUPS]
            S2g, MXg, E2g, SMg, RSg, ALPHg = [], [], [], [], [], []
            for gi, nb in enumerate(GN):
                s2t = smallp.tile([nb, L], fp32, name=f"s2_{gi}")
                mxt = smallp.tile([nb, 1], fp32, name=f"mx_{gi}")
                e2t = smallp.tile([nb, L], fp32, name=f"e2_{gi}")
                smt = smallp.tile([nb, 1], fp32, name=f"sm_{gi}")
                rst = smallp.tile([nb, 1], fp32, name=f"rs_{gi}")
                alt = smallp.tile([nb, L], fp32, name=f"al_{gi}")
                S2g.append(s2t); MXg.append(mxt); E2g.append(e2t)
                SMg.append(smt); RSg.append(rst); ALPHg.append(alt)
            B2G = {}
            for gi, (bs, _) in enumerate(B_GROUPS):
                for i, b in enumerate(bs):
                    B2G[b] = (gi, i)
            CTX = smallp.tile([BPC, E], fp32)
            n_et = E // TW
            ps_c = []
            for et in range(n_et):
                ps_ct = psctx.tile([BPC, TW], fp32, tag=f"ctx{et}", name=f"psc{et}")
                ps_c.append(ps_ct)
            xn_tiles = {}
            ctx_chunk = [0]  # running index over the 32 (b, j) context chunks

            def do_group(gi):
                bs, _ = B_GROUPS[gi]
                nb = len(bs)
                b0, b1 = bs[0], bs[-1] + 1
                S2, MX, E2, SM, RS, ALPH = (
                    S2g[gi], MXg[gi], E2g[gi], SMg[gi], RSg[gi], ALPHg[gi])
                nc.vector.tensor_reduce(
                    MX[:], S2[:], axis=mybir.AxisListType.X,
                    op=mybir.AluOpType.max, negate=True,
                )
                nc.scalar.activation(
                    E2[:], S2[:], mybir.ActivationFunctionType.Exp,
                    bias=MX[:, 0:1], scale=1.0, accum_out=SM[:, 0:1],
                )
                nc.vector.reciprocal(RS[:, 0:1], SM[:, 0:1])
                nc.vector.tensor_scalar_mul(ALPH[:], E2[:], RS[:, 0:1])
                nc.sync.dma_start(alpha_o[b0:b1, :], ALPH[:])
                # transpose this group's alpha rows on PE: [nb, l] -> [l, nb]
                TA = pssc.tile([128, 2 * BPC], fp32, tag="sc", name=f"ta{gi}")
                nc.tensor.matmul(TA[:, 0:nb], ALPH[:, 0:128], ID16[0:nb, 0:nb],
                                 is_transpose=True, skip_group_check=True)
                nc.tensor.matmul(TA[0:L - 128, BPC:BPC + nb], ALPH[:, 128:L],
                                 ID16[0:nb, 0:nb], is_transpose=True,
                                 skip_group_check=True)
                for i, b in enumerate(bs):
                    m0 = atp.tile([128, BPC], bf16, tag=f"at0_{b}", name=f"m0_{b}")
                    nc.vector.tensor_mul(
                        m0[:], TA[:, i:i + 1].broadcast_to((128, BPC)),
                        MSK[:, b * BPC:(b + 1) * BPC])
                    m1 = atp.tile([L - 128, BPC], bf16, tag=f"at1_{b}",
                                  name=f"m1_{b}")
                    nc.vector.tensor_mul(
                        m1[:],
                        TA[0:L - 128, BPC + i:BPC + i + 1].broadcast_to(
                            (L - 128, BPC)),
                        MSK[0:L - 128, b * BPC:(b + 1) * BPC])
                    for j, lhs in ((0, m0), (1, m1)):
                        rhs_tile = xn_tiles[b][j]
                        idx = ctx_chunk[0]
                        ctx_chunk[0] += 1
                        for et in range(n_et):
                            nc.tensor.matmul(
                                ps_c[et][:],
                                lhs[:],
                                rhs_tile[:, et * TW:(et + 1) * TW],
                                start=(idx == 0),
                                stop=(idx == 2 * BPC - 1),
                            )

            # ---- stage 1: main matmul + tanh + scores over t-tiles,
            #      with xn loads and group softmax/context interleaved ----
            group_after = {g_tile: gi for gi, (_, g_tile) in enumerate(B_GROUPS)}
            for ti, (t0, w) in enumerate(T_TILES):
                XTt = xtp.tile([128, EC, TW], bf16, tag="xt")
                nc.sync.dma_start(XTt[:, :, :w], xt[:, :, t0:t0 + w])
                for b in XN_SCHED.get(ti, []):
                    t_lo = b * L
                    xa = xnp.tile([128, E], bf16, tag=f"xn0_{b}", name=f"xa{b}")
                    nc.sync.dma_start(xa[:], xn[t_lo:t_lo + 128, :])
                    xb = xnp.tile([L - 128, E], bf16, tag=f"xn1_{b}", name=f"xb{b}")
                    nc.sync.dma_start(xb[:], xn[t_lo + 128:t_lo + L, :])
                    xn_tiles[b] = (xa, xb)
                CMBt = cmbp.tile([128, AT, TW], bf16, tag="cmb")
                for at in range(AT):
                    ps = psmm.tile([128, TW], fp32, tag="mm")
                    for ec in range(EC):
                        nc.tensor.matmul(
                            ps[:, :w],
                            W2s[:, ec, at * 128:(at + 1) * 128],
                            XTt[:, ec, :w],
                            start=(ec == 0),
                            stop=(ec == EC - 1),
                        )
                    for (b, s0, s1) in _segments(t0, w):
                        nc.scalar.activation(
                            CMBt[:, at, s0:s1],
                            ps[:, s0:s1],
                            mybir.ActivationFunctionType.Tanh,
                            bias=w1b[:, at, b:b + 1],
                            scale=1.0,
                        )
                ps_s = pssc.tile([1, TW], fp32, tag="sc")
                for at in range(AT):
                    nc.tensor.matmul(
                        ps_s[:, :w],
                        AW[:, at:at + 1],
                        CMBt[:, at, :w],
                        start=(at == 0),
                        stop=(at == AT - 1),
                    )
                SCRt = cmbp.tile([1, TW], fp32, tag="scr")
                nc.vector.tensor_copy(SCRt[0:1, :w], ps_s[0:1, :w])
                for (b, s0, s1) in _segments(t0, w):
                    l0 = t0 + s0 - b * L
                    gi, gr = B2G[b]
                    nc.sync.dma_start(
                        S2g[gi][gr:gr + 1, l0:l0 + (s1 - s0)], SCRt[0:1, s0:s1])
                if ti in group_after:
                    do_group(group_after[ti])

            # ---- epilogue: context out ----
            for et in range(n_et):
                nc.vector.tensor_copy(CTX[:, et * TW:(et + 1) * TW], ps_c[et][:])
            nc.sync.dma_start(ctx_o[:], CTX[:])

    nc.finalize()
    return nc


_PROGRAM = None


def _get_program():
    global _PROGRAM
    if _PROGRAM is None:
        _PROGRAM = _build_program()
    return _PROGRAM


def _host_prep(features, hidden_state, W1_w, W1_b, W2_w, W2_b, A_w, A_b):
    """Build the per-core input maps (all layout prep on host)."""
    feats = np.ascontiguousarray(features.reshape(N_CORES, BPC, L, E))
    hids = hidden_state.reshape(N_CORES, BPC, D)

    w2s = np.ascontiguousarray(
        W2_w.reshape(EC, 128, A).transpose(1, 0, 2)).astype(BF16)
    w1s = np.ascontiguousarray(W1_w.reshape(DC, 128, A).transpose(1, 0, 2)).astype(F32)
    b12v = (W1_b + W2_b).astype(F32).reshape(AT, 128).T.copy()
    awv = A_w.astype(F32).reshape(AT, 128).T.astype(BF16).copy()
    mskv = np.zeros((128, BPC * BPC), dtype=BF16)
    for b in range(BPC):
        mskv[:, b * BPC + b] = 1.0
    id16v = np.eye(BPC, dtype=F32)

    in_maps = []
    for c in range(N_CORES):
        f = feats[c].reshape(T, E)
        xnv = f.astype(BF16)
        xtv = np.ascontiguousarray(f.T).astype(BF16).reshape(EC, 128, T)
        xtv = np.ascontiguousarray(xtv.transpose(1, 0, 2))
        hidv = np.ascontiguousarray(
            hids[c].T.reshape(DC, 128, BPC).transpose(1, 0, 2)).astype(F32)
        in_maps.append({
            "xt": xtv, "xn": xnv, "w2s": w2s, "w1s": w1s, "hid": hidv,
            "b12": b12v, "aw": awv, "msk": mskv, "id16": id16v,
        })
    return in_maps


def run(in_maps, trace=False, **kw):
    nc = _get_program()
    return run_bass_kernel_spmd(nc, in_maps, list(range(N_CORES)), trace=trace, **kw)


def kernel(features, hidden_state, W1_w, W1_b, W2_w, W2_b, A_w, A_b):
    features = np.asarray(features, dtype=F32)
    hidden_state = np.asarray(hidden_state, dtype=F32)
    in_maps = _host_prep(
        features, hidden_state, np.asarray(W1_w), np.asarray(W1_b),
        np.asarray(W2_w), np.asarray(W2_b), np.asarray(A_w), np.asarray(A_b))
    res = run(in_maps, trace=False)
    alpha = np.concatenate([res.results[c]["alpha"] for c in range(N_CORES)], axis=0)
    context = np.concatenate([res.results[c]["ctx"] for c in range(N_CORES)], axis=0)
    return alpha.reshape(B_FULL, L), context.reshape(B_FULL, E)
